# revision 22
# baseline (speedup 1.0000x reference)
"""Trainium2 Bass kernel for nn_AttractorState — sequence-parallel sharding.

Reference computation (per batch b):
    C[b] = sum_t alpha^(S-1-t) * (W @ h_t + bias) outer e_t        (S = 8192)

Refactored:
    G[b]  = H[b]^T @ (w . PE[b])          # [d_model=512, d_model=512], w_t = alpha^(S-1-t)
    C[b]  = W @ G[b]  (+ bias outer r)    # [d_state=512, d_model=512]

Sharding over 8 NeuronCores, per the sequence-parallel hint:
(batch=4) x (seq-half=2).  Core (b, j) owns tokens [j*4096, (j+1)*4096)
at full width and computes the partial state
    C_j[b] = W @ (H_j^T @ (w_j . PE_j))   # [512, 512]
with the GLOBAL decay weights w_t baked in (each shard uses its own global
w slice, so partials combine by plain addition — the decay-scaled
all-reduce of the hint).  The two partials per batch are summed during
host-side unsharding (the pair-sum IS the unshard for sum-sharded
partials; on-device NRT collectives measure 40-70us of entry/serialize
overhead on this 8-core setup — 10x the data cost — so the 1MB/pair
reduction rides the output gather instead).

Per-core HBM traffic: 8.5 MiB (hp interleaved h+decayed-pe rows in bf16,
W^T, bf16 partial out).  G[e,d] accumulates with e on partitions so the
tail needs no transposes.

Schedule notes (v3, from NTFF profile analysis):
- All DRAM tensors are host-staged PARTITION-MAJOR ([128, ...] with each
  partition's bytes contiguous), so every DMA is one large contiguous
  descriptor per partition.  The HWDGE generates descriptors at ~10ns
  each; with row-wise (2 KiB) descriptors that caps a ring at ~200 GB/s
  and dominates the ramp.  One descriptor per partition per chunk makes
  descriptor generation negligible.
- The PE HAM clock gate starts at K=4/8 (1.2 GHz) and reaches 2.4 GHz
  only after ~3.4us of sustained matmul activity.  Six warmup matmuls on
  a vector-memset scratch tile (into a PSUM bank later reused for C)
  run during the DMA ramp, so data matmuls execute at the warm
  ~216 ns/MM N=512 streaming rate from the first tile.
- hp rides in 13 triggers: singles/pairs up front (low latency while the
  rings ramp), then 4-tile chunks.
- W@G runs ce-major so it only needs g_sb[ce] as the PE reaches round ce
  — no PE stall on the g3 PSUM->SBUF copy.
- PSUM->SBUF copies alternate vector/scalar; output slices pair up into
  two DMAs on alternating queues so only the last 256 KiB is exposed.

The bias path (b != 0) needs an extra rank-1 accumulation r = w^T @ PE and
an outer-product matmul; setup_inputs() always produces b == 0, so the
default graph skips it and a bias-capable graph is built only if a nonzero
b ever shows up.
"""

import math
import sys

import numpy as np

for _p in ("/opt/trn_rl_repo", "/opt/trn_rl_repo/concourse"):
    if _p not in sys.path:
        sys.path.append(_p)

# Problem constants (hardcoded per harness contract).
B = 4
S = 8192
D = 512          # d_model
E = 512          # d_state
P = 128          # SBUF partitions
NCORES = 8
SH = S // 2      # 4096 tokens per core
NT = SH // P     # 32 t-tiles per core
HP = 2 * D       # 1024, interleaved h+pew row

# hp chunk schedule: (start, width) pairs, alternating sync/scalar queues.
# (start, width, ring): the two HWDGE rings (sync/scalar) alternate;
# total SDMA/HBM bandwidth (~390 GB/s steady, less in the ramp) does not
# grow with more rings, so a third (SWDGE) ring only splits it.
CHUNKS = [
    (0, 1, "sync"), (1, 1, "scalar"), (2, 2, "sync"), (4, 2, "scalar"),
    (6, 2, "sync"), (8, 2, "scalar"), (10, 2, "sync"), (12, 2, "scalar"),
    (14, 4, "sync"), (18, 4, "scalar"), (22, 4, "sync"), (26, 4, "scalar"),
    (30, 2, "sync"),
]
assert sum(w for _, w, _e in CHUNKS) == NT
assert [s for s, _, _e in CHUNKS] == [
    sum(w for _, w, _e in CHUNKS[:i]) for i in range(len(CHUNKS))
]
WL = CHUNKS[-1][1]   # last chunk is k-grouped for staggered g finalization

N_WARMUP = 24    # N=128 warmup matmuls: flips the HAM gate during the DMA ramp
# Pacing matmuls interleaved after early tiles: warm-rate consumption
# (303 GB/s) outruns the DMA ramp (~280-350 GB/s effective), so a few
# ~56-107ns scratch matmuls after each early tile slow demand to match
# supply instead of risking us-scale starvation stalls.
PACE = {0: 3, 1: 3, 2: 3, 3: 3, 4: 2, 5: 2, 6: 2, 7: 2, 8: 2, 9: 2,
        10: 1, 11: 1, 12: 1, 13: 1}

_GRAPH_CACHE = {}


def _decay_weights():
    # Match reference: alpha = f32(exp(-pi/S)); w = exp((S-1-t) * log(alpha)) in f32.
    alpha = np.float32(math.exp(-math.pi / S))
    t = np.arange(S, dtype=np.float32)
    w = np.exp((np.float32(S - 1.0) - t) * np.log(alpha)).astype(np.float32)
    return w


def _build(bias: bool):
    key = "bias" if bias else "nobias"
    if key in _GRAPH_CACHE:
        return _GRAPH_CACHE[key]

    import concourse.bass as bass  # noqa: F401
    import concourse.mybir as mybir
    import concourse.tile as tile
    from concourse import bacc

    f32 = mybir.dt.float32
    bf16 = mybir.dt.bfloat16

    nc = bacc.Bacc("TRN2", target_bir_lowering=False)

    # All DRAM tensors partition-major: per-partition bytes contiguous.
    hp_ext = nc.declare_dram_parameter("hp", [P, NT, HP], bf16, isOutput=False)
    wt_ext = nc.declare_dram_parameter("wt", [P, 4, E], bf16, isOutput=False)
    b_ext = nc.declare_dram_parameter("b", [E], f32, isOutput=False)
    out_ext = nc.declare_dram_parameter("out", [P, 4, D], bf16, isOutput=True)

    with tile.TileContext(nc) as tc:
        with (
            tc.tile_pool(name="sb", bufs=1) as io,
            tc.tile_pool(name="acc", bufs=1, space="PSUM") as acc_pool,
        ):
            consts = io

            # ---- PE warmup: matmuls on a memset scratch tile into a PSUM
            # bank later reused for C (never read from the warmup).  Keeps
            # the PE busy through the HAM activity window while the hp
            # DMAs ramp, so the data matmuls all run at 2.4 GHz. ----
            warm_sb = io.tile([P, P], bf16, tag="warm_sb")
            nc.vector.memset(warm_sb[:], 0.0)
            warm_ps = acc_pool.tile([P, P], f32, tag="cps0", name="warm")
            for i in range(N_WARMUP):
                nc.tensor.matmul(
                    warm_ps[:],
                    warm_sb[:],
                    warm_sb[:],
                    start=(i == 0),
                    stop=(i == N_WARMUP - 1),
                )

            # ---- input stream: one resident hp slab, chunked DMA ----
            hp_t = io.tile([P, NT, HP], bf16)
            rings = {"sync": nc.sync, "scalar": nc.scalar, "gpsimd": nc.gpsimd}
            for n0, w, ring in CHUNKS:
                rings[ring].dma_start(
                    hp_t[:, n0:n0 + w, :], hp_ext.ap()[:, n0:n0 + w, :]
                )

            # ---- constants (sync queue tail; needed only at the end) ----
            wt_sb = consts.tile([P, 4, E], bf16)   # wt_sb[p,c,s] = W[s, c*128+p]
            nc.sync.dma_start(wt_sb[:], wt_ext.ap())
            if bias:
                b_sb = consts.tile([1, E], f32)
                nc.sync.dma_start(b_sb[:], b_ext.ap().unsqueeze(0))
                b_bf = consts.tile([1, E], bf16)
                nc.vector.tensor_copy(b_bf[:], b_sb[:])
                ones_sb = consts.tile([P, 1], bf16)
                nc.vector.memset(ones_sb[:], 1.0)

            # ---- G (/ r) accumulation over this core's 4096 tokens ----
            # G[e, d] += h[t, e-slice]^T @ pew[t, :], e on partitions
            g_ps = [
                acc_pool.tile([P, E], f32, tag=f"g{k}", name=f"g{k}")
                for k in range(4)
            ]
            if bias:
                r_ps = acc_pool.tile([1, E], f32, tag="r")

            for n in range(NT - WL):
                for k in range(4):
                    nc.tensor.matmul(
                        g_ps[k][:],
                        hp_t[:, n, k * P:(k + 1) * P],
                        hp_t[:, n, D:HP],
                        start=(n == 0),
                        stop=False,
                    )
                if bias:
                    nc.tensor.matmul(
                        r_ps[:],
                        ones_sb[:],
                        hp_t[:, n, D:HP],
                        start=(n == 0),
                        stop=False,
                    )
                for _ in range(PACE.get(n, 0)):
                    nc.tensor.matmul(
                        warm_ps[:], warm_sb[:], warm_sb[:],
                        start=True, stop=True,
                    )
            # final chunk k-grouped: g_ps[k] finalize in order so their
            # PSUM->SBUF copies overlap the remaining matmuls
            nL = NT - WL
            for k in range(4):
                for i in range(WL):
                    nc.tensor.matmul(
                        g_ps[k][:],
                        hp_t[:, nL + i, k * P:(k + 1) * P],
                        hp_t[:, nL + i, D:HP],
                        start=False,
                        stop=(i == WL - 1),
                    )
            if bias:
                for i in range(WL):
                    nc.tensor.matmul(
                        r_ps[:],
                        ones_sb[:],
                        hp_t[:, nL + i, D:HP],
                        start=False,
                        stop=(i == WL - 1),
                    )

            # ---- G -> SBUF (bf16); no transposes needed ----
            g_sb = consts.tile([P, 4, E], bf16)   # g_sb[p,c,d] = G[c*128+p, d]
            for k in range(4):
                nc.vector.tensor_copy(g_sb[:, k, :], g_ps[k][:])
            if bias:
                rred_bf = consts.tile([1, E], bf16)
                nc.vector.tensor_copy(rred_bf[:], r_ps[:])

            # ---- partial C = W @ G (+ b outer r), bf16 out ----
            # ce-major: round ce only needs g_sb[ce], so the PE never
            # waits on the tail PSUM->SBUF copies.
            c_ps = [
                acc_pool.tile([P, E], f32, tag=f"cps{cs}", name=f"cps{cs}")
                for cs in range(4)
            ]
            # final round ordered [2,3,0,1] so the hi slab's copies and DMA
            # launch first and the two output transfers overlap
            for ce in range(4):
                cs_order = [0, 1, 2, 3] if ce < 3 else [2, 3, 0, 1]
                for cs in cs_order:
                    nc.tensor.matmul(
                        c_ps[cs][:],
                        wt_sb[:, ce, cs * P:(cs + 1) * P],
                        g_sb[:, ce, :],
                        start=(ce == 0),
                        stop=(not bias and ce == 3),
                    )
            if bias:
                for cs in range(4):
                    nc.tensor.matmul(
                        c_ps[cs][:],
                        b_bf[0:1, cs * P:(cs + 1) * P],
                        rred_bf[:],
                        start=False,
                        stop=True,
                    )
            c_lo = io.tile([P, 2, E], bf16, tag="clo")
            c_hi = io.tile([P, 2, E], bf16, tag="chi")
            nc.vector.tensor_copy(c_hi[:, 0, :], c_ps[2][:])
            nc.scalar.activation(
                c_hi[:, 1, :], c_ps[3][:], mybir.ActivationFunctionType.Copy
            )
            nc.scalar.dma_start(out_ext.ap()[:, 2:4, :], c_hi[:])
            nc.vector.tensor_copy(c_lo[:, 0, :], c_ps[0][:])
            nc.scalar.activation(
                c_lo[:, 1, :], c_ps[1][:], mybir.ActivationFunctionType.Copy
            )
            nc.sync.dma_start(out_ext.ap()[:, 0:2, :], c_lo[:])

    nc.compile()
    _GRAPH_CACHE[key] = nc
    return nc


def _in_maps(hidden_states, positional_encodings, W, b):
    import ml_dtypes

    bf16 = ml_dtypes.bfloat16
    w_full = _decay_weights()[:, None]  # constant decay, folded into pe staging
    # wt partition-major: wt[p, c, s] = W[s, c*128+p]
    wt = np.ascontiguousarray(
        np.asarray(W, dtype=np.float32).T.astype(bf16)
        .reshape(4, P, E).transpose(1, 0, 2)
    )
    b_c = np.ascontiguousarray(b, dtype=np.float32)
    maps = []
    for c in range(NCORES):
        bi, sj = c // 2, c % 2
        lo, hi = sj * SH, (sj + 1) * SH
        # hp partition-major: hp[p, n, 0:D] = h[n*128+p], hp[p, n, D:] = pew[n*128+p]
        hp = np.empty((P, NT, HP), dtype=bf16)
        hp[:, :, 0:D] = (
            np.asarray(hidden_states[bi, lo:hi], dtype=np.float32)
            .astype(bf16).reshape(NT, P, D).transpose(1, 0, 2)
        )
        hp[:, :, D:HP] = (
            (np.asarray(positional_encodings[bi, lo:hi], dtype=np.float32)
             * w_full[lo:hi])
            .astype(bf16).reshape(NT, P, D).transpose(1, 0, 2)
        )
        maps.append({"hp": hp, "wt": wt, "b": b_c})
    return maps


def _assemble(results):
    # pair-sum is the unshard for sum-sharded partial states;
    # out is partition-major: out[p, cs, d] = C[cs*128+p, d]
    out = np.empty((B, E, D), dtype=np.float32)
    for bi in range(B):
        c = results[2 * bi]["out"].astype(np.float32) + results[
            2 * bi + 1
        ]["out"].astype(np.float32)
        out[bi] = c.transpose(1, 0, 2).reshape(E, D)
    return out


def run(hidden_states, positional_encodings, W, b, trace=False, **trace_kwargs):
    from concourse.bass_utils import run_bass_kernel_spmd

    nc = _build(bias=bool(np.any(np.asarray(b) != 0)))
    maps = _in_maps(hidden_states, positional_encodings, W, b)
    res = run_bass_kernel_spmd(
        nc, maps, core_ids=list(range(NCORES)), trace=trace, **trace_kwargs
    )
    return _assemble(res.results), res


def kernel(hidden_states, positional_encodings, W, b):
    out, _ = run(hidden_states, positional_encodings, W, b, trace=False)
    return out


# revision 23
# speedup vs baseline: 1.1233x; 1.1233x over previous
"""Trainium2 Bass kernel for nn_AttractorState — sequence-parallel sharding.

Reference computation (per batch b):
    C[b] = sum_t alpha^(S-1-t) * (W @ h_t + bias) outer e_t        (S = 8192)

Refactored:
    G[b]  = H[b]^T @ (w . PE[b])          # [d_model=512, d_model=512], w_t = alpha^(S-1-t)
    C[b]  = W @ G[b]  (+ bias outer r)    # [d_state=512, d_model=512]

Sharding over 8 NeuronCores, per the sequence-parallel hint:
(batch=4) x (seq-half=2).  Core (b, j) owns tokens [j*4096, (j+1)*4096)
at full width and computes the partial state
    C_j[b] = W @ (H_j^T @ (w_j . PE_j))   # [512, 512]
with the GLOBAL decay weights w_t baked in (each shard uses its own global
w slice, so partials combine by plain addition — the decay-scaled
all-reduce of the hint).  The two partials per batch are summed during
host-side unsharding (the pair-sum IS the unshard for sum-sharded
partials; on-device NRT collectives measure 40-70us of entry/serialize
overhead on this 8-core setup — 10x the data cost — so the 1MB/pair
reduction rides the output gather instead).

Per-core HBM traffic: 8.5 MiB (hp interleaved h+decayed-pe rows in bf16,
W^T, bf16 partial out).  G[e,d] accumulates with e on partitions so the
tail needs no transposes.

Schedule notes (v3, from NTFF profile analysis):
- All DRAM tensors are host-staged PARTITION-MAJOR ([128, ...] with each
  partition's bytes contiguous), so every DMA is one large contiguous
  descriptor per partition.  The HWDGE generates descriptors at ~10ns
  each; with row-wise (2 KiB) descriptors that caps a ring at ~200 GB/s
  and dominates the ramp.  One descriptor per partition per chunk makes
  descriptor generation negligible.
- The PE HAM clock gate starts at K=4/8 (1.2 GHz) and reaches 2.4 GHz
  only after ~3.4us of sustained matmul activity.  Six warmup matmuls on
  a vector-memset scratch tile (into a PSUM bank later reused for C)
  run during the DMA ramp, so data matmuls execute at the warm
  ~216 ns/MM N=512 streaming rate from the first tile.
- hp rides in 13 triggers: singles/pairs up front (low latency while the
  rings ramp), then 4-tile chunks.
- W@G runs ce-major so it only needs g_sb[ce] as the PE reaches round ce
  — no PE stall on the g3 PSUM->SBUF copy.
- PSUM->SBUF copies alternate vector/scalar; output slices pair up into
  two DMAs on alternating queues so only the last 256 KiB is exposed.

The bias path (b != 0) needs an extra rank-1 accumulation r = w^T @ PE and
an outer-product matmul; setup_inputs() always produces b == 0, so the
default graph skips it and a bias-capable graph is built only if a nonzero
b ever shows up.
"""

import math
import sys

import numpy as np

for _p in ("/opt/trn_rl_repo", "/opt/trn_rl_repo/concourse"):
    if _p not in sys.path:
        sys.path.append(_p)

# Problem constants (hardcoded per harness contract).
B = 4
S = 8192
D = 512          # d_model
E = 512          # d_state
P = 128          # SBUF partitions
NCORES = 8
SH = S // 2      # 4096 tokens per core
NT = SH // P     # 32 t-tiles per core
HP = 2 * D       # 1024, interleaved h+pew row

# hp chunk schedule: (start, width) pairs, alternating sync/scalar queues.
# (start, width, ring): the two HWDGE rings (sync/scalar) alternate;
# total SDMA/HBM bandwidth (~390 GB/s steady, less in the ramp) does not
# grow with more rings, so a third (SWDGE) ring only splits it.
CHUNKS = [
    (0, 1, "sync"), (1, 1, "scalar"), (2, 2, "sync"), (4, 2, "scalar"),
    (6, 2, "sync"), (8, 2, "scalar"), (10, 2, "sync"), (12, 2, "scalar"),
    (14, 4, "sync"), (18, 4, "scalar"), (22, 4, "sync"), (26, 4, "scalar"),
    (30, 2, "sync"),
]
assert sum(w for _, w, _e in CHUNKS) == NT
assert [s for s, _, _e in CHUNKS] == [
    sum(w for _, w, _e in CHUNKS[:i]) for i in range(len(CHUNKS))
]
WL = CHUNKS[-1][1]   # last chunk is k-grouped for staggered g finalization

N_WARMUP = 6     # ~3us of cold-rate warmup matmuls to flip the HAM gate

_GRAPH_CACHE = {}


def _decay_weights():
    # Match reference: alpha = f32(exp(-pi/S)); w = exp((S-1-t) * log(alpha)) in f32.
    alpha = np.float32(math.exp(-math.pi / S))
    t = np.arange(S, dtype=np.float32)
    w = np.exp((np.float32(S - 1.0) - t) * np.log(alpha)).astype(np.float32)
    return w


def _build(bias: bool):
    key = "bias" if bias else "nobias"
    if key in _GRAPH_CACHE:
        return _GRAPH_CACHE[key]

    import concourse.bass as bass  # noqa: F401
    import concourse.mybir as mybir
    import concourse.tile as tile
    from concourse import bacc

    f32 = mybir.dt.float32
    bf16 = mybir.dt.bfloat16

    nc = bacc.Bacc("TRN2", target_bir_lowering=False)

    # All DRAM tensors partition-major: per-partition bytes contiguous.
    hp_ext = nc.declare_dram_parameter("hp", [P, NT, HP], bf16, isOutput=False)
    wt_ext = nc.declare_dram_parameter("wt", [P, 4, E], bf16, isOutput=False)
    b_ext = nc.declare_dram_parameter("b", [E], f32, isOutput=False)
    out_ext = nc.declare_dram_parameter("out", [P, 4, D], bf16, isOutput=True)

    with tile.TileContext(nc) as tc:
        with (
            tc.tile_pool(name="sb", bufs=1) as io,
            tc.tile_pool(name="acc", bufs=1, space="PSUM") as acc_pool,
        ):
            consts = io

            # ---- PE warmup: matmuls on a memset scratch tile into a PSUM
            # bank later reused for C (never read from the warmup).  Keeps
            # the PE busy through the HAM activity window while the hp
            # DMAs ramp, so the data matmuls all run at 2.4 GHz. ----
            warm_sb = io.tile([P, E], bf16, tag="warm_sb")
            nc.vector.memset(warm_sb[:], 0.0)
            warm_ps = acc_pool.tile([P, E], f32, tag="cps0", name="warm")
            for i in range(N_WARMUP):
                nc.tensor.matmul(
                    warm_ps[:],
                    warm_sb[:, 0:P],
                    warm_sb[:],
                    start=(i == 0),
                    stop=(i == N_WARMUP - 1),
                )

            # ---- input stream: one resident hp slab, chunked DMA ----
            hp_t = io.tile([P, NT, HP], bf16)
            rings = {"sync": nc.sync, "scalar": nc.scalar, "gpsimd": nc.gpsimd}
            for n0, w, ring in CHUNKS:
                rings[ring].dma_start(
                    hp_t[:, n0:n0 + w, :], hp_ext.ap()[:, n0:n0 + w, :]
                )

            # ---- constants (sync queue tail; needed only at the end) ----
            wt_sb = consts.tile([P, 4, E], bf16)   # wt_sb[p,c,s] = W[s, c*128+p]
            nc.sync.dma_start(wt_sb[:], wt_ext.ap())
            if bias:
                b_sb = consts.tile([1, E], f32)
                nc.sync.dma_start(b_sb[:], b_ext.ap().unsqueeze(0))
                b_bf = consts.tile([1, E], bf16)
                nc.vector.tensor_copy(b_bf[:], b_sb[:])
                ones_sb = consts.tile([P, 1], bf16)
                nc.vector.memset(ones_sb[:], 1.0)

            # ---- G (/ r) accumulation over this core's 4096 tokens ----
            # G[e, d] += h[t, e-slice]^T @ pew[t, :], e on partitions
            g_ps = [
                acc_pool.tile([P, E], f32, tag=f"g{k}", name=f"g{k}")
                for k in range(4)
            ]
            if bias:
                r_ps = acc_pool.tile([1, E], f32, tag="r")

            for n in range(NT - WL):
                for k in range(4):
                    nc.tensor.matmul(
                        g_ps[k][:],
                        hp_t[:, n, k * P:(k + 1) * P],
                        hp_t[:, n, D:HP],
                        start=(n == 0),
                        stop=False,
                    )
                if bias:
                    nc.tensor.matmul(
                        r_ps[:],
                        ones_sb[:],
                        hp_t[:, n, D:HP],
                        start=(n == 0),
                        stop=False,
                    )
            # final chunk k-grouped: g_ps[k] finalize in order so their
            # PSUM->SBUF copies overlap the remaining matmuls
            nL = NT - WL
            for k in range(4):
                for i in range(WL):
                    nc.tensor.matmul(
                        g_ps[k][:],
                        hp_t[:, nL + i, k * P:(k + 1) * P],
                        hp_t[:, nL + i, D:HP],
                        start=False,
                        stop=(i == WL - 1),
                    )
            if bias:
                for i in range(WL):
                    nc.tensor.matmul(
                        r_ps[:],
                        ones_sb[:],
                        hp_t[:, nL + i, D:HP],
                        start=False,
                        stop=(i == WL - 1),
                    )

            # ---- G -> SBUF (bf16); no transposes needed ----
            g_sb = consts.tile([P, 4, E], bf16)   # g_sb[p,c,d] = G[c*128+p, d]
            for k in range(4):
                nc.vector.tensor_copy(g_sb[:, k, :], g_ps[k][:])
            if bias:
                rred_bf = consts.tile([1, E], bf16)
                nc.vector.tensor_copy(rred_bf[:], r_ps[:])

            # ---- partial C = W @ G (+ b outer r), bf16 out ----
            # ce-major: round ce only needs g_sb[ce], so the PE never
            # waits on the tail PSUM->SBUF copies.
            c_ps = [
                acc_pool.tile([P, E], f32, tag=f"cps{cs}", name=f"cps{cs}")
                for cs in range(4)
            ]
            for ce in range(4):
                for cs in range(4):
                    nc.tensor.matmul(
                        c_ps[cs][:],
                        wt_sb[:, ce, cs * P:(cs + 1) * P],
                        g_sb[:, ce, :],
                        start=(ce == 0),
                        stop=(not bias and ce == 3),
                    )
            if bias:
                for cs in range(4):
                    nc.tensor.matmul(
                        c_ps[cs][:],
                        b_bf[0:1, cs * P:(cs + 1) * P],
                        rred_bf[:],
                        start=False,
                        stop=True,
                    )
            c_lo = io.tile([P, 2, E], bf16, tag="clo")
            c_hi = io.tile([P, 2, E], bf16, tag="chi")
            nc.vector.tensor_copy(c_lo[:, 0, :], c_ps[0][:])
            nc.scalar.activation(
                c_lo[:, 1, :], c_ps[1][:], mybir.ActivationFunctionType.Copy
            )
            nc.sync.dma_start(out_ext.ap()[:, 0:2, :], c_lo[:])
            nc.vector.tensor_copy(c_hi[:, 0, :], c_ps[2][:])
            nc.scalar.activation(
                c_hi[:, 1, :], c_ps[3][:], mybir.ActivationFunctionType.Copy
            )
            nc.scalar.dma_start(out_ext.ap()[:, 2:4, :], c_hi[:])

    nc.compile()
    _GRAPH_CACHE[key] = nc
    return nc


def _in_maps(hidden_states, positional_encodings, W, b):
    import ml_dtypes

    bf16 = ml_dtypes.bfloat16
    w_full = _decay_weights()[:, None]  # constant decay, folded into pe staging
    # wt partition-major: wt[p, c, s] = W[s, c*128+p]
    wt = np.ascontiguousarray(
        np.asarray(W, dtype=np.float32).T.astype(bf16)
        .reshape(4, P, E).transpose(1, 0, 2)
    )
    b_c = np.ascontiguousarray(b, dtype=np.float32)
    maps = []
    for c in range(NCORES):
        bi, sj = c // 2, c % 2
        lo, hi = sj * SH, (sj + 1) * SH
        # hp partition-major: hp[p, n, 0:D] = h[n*128+p], hp[p, n, D:] = pew[n*128+p]
        hp = np.empty((P, NT, HP), dtype=bf16)
        hp[:, :, 0:D] = (
            np.asarray(hidden_states[bi, lo:hi], dtype=np.float32)
            .astype(bf16).reshape(NT, P, D).transpose(1, 0, 2)
        )
        hp[:, :, D:HP] = (
            (np.asarray(positional_encodings[bi, lo:hi], dtype=np.float32)
             * w_full[lo:hi])
            .astype(bf16).reshape(NT, P, D).transpose(1, 0, 2)
        )
        maps.append({"hp": hp, "wt": wt, "b": b_c})
    return maps


def _assemble(results):
    # pair-sum is the unshard for sum-sharded partial states;
    # out is partition-major: out[p, cs, d] = C[cs*128+p, d]
    out = np.empty((B, E, D), dtype=np.float32)
    for bi in range(B):
        c = results[2 * bi]["out"].astype(np.float32) + results[
            2 * bi + 1
        ]["out"].astype(np.float32)
        out[bi] = c.transpose(1, 0, 2).reshape(E, D)
    return out


def run(hidden_states, positional_encodings, W, b, trace=False, **trace_kwargs):
    from concourse.bass_utils import run_bass_kernel_spmd

    nc = _build(bias=bool(np.any(np.asarray(b) != 0)))
    maps = _in_maps(hidden_states, positional_encodings, W, b)
    res = run_bass_kernel_spmd(
        nc, maps, core_ids=list(range(NCORES)), trace=trace, **trace_kwargs
    )
    return _assemble(res.results), res


def kernel(hidden_states, positional_encodings, W, b):
    out, _ = run(hidden_states, positional_encodings, W, b, trace=False)
    return out


# revision 25
# speedup vs baseline: 1.2006x; 1.0688x over previous
"""Trainium2 Bass kernel for nn_AttractorState — sequence-parallel sharding.

Reference computation (per batch b):
    C[b] = sum_t alpha^(S-1-t) * (W @ h_t + bias) outer e_t        (S = 8192)

Refactored:
    G[b]  = H[b]^T @ (w . PE[b])          # [d_model=512, d_model=512], w_t = alpha^(S-1-t)
    C[b]  = W @ G[b]  (+ bias outer r)    # [d_state=512, d_model=512]

Sharding over 8 NeuronCores, per the sequence-parallel hint:
(batch=4) x (seq-half=2).  Core (b, j) owns tokens [j*4096, (j+1)*4096)
at full width and computes the partial state
    C_j[b] = W @ (H_j^T @ (w_j . PE_j))   # [512, 512]
with the GLOBAL decay weights w_t baked in (each shard uses its own global
w slice, so partials combine by plain addition — the decay-scaled
all-reduce of the hint).  The two partials per batch are summed during
host-side unsharding (the pair-sum IS the unshard for sum-sharded
partials; on-device NRT collectives measure 40-70us of entry/serialize
overhead on this 8-core setup — 10x the data cost — so the 1MB/pair
reduction rides the output gather instead).

Per-core HBM traffic: 8.5 MiB (hp interleaved h+decayed-pe rows in bf16,
W^T, bf16 partial out).  G[e,d] accumulates with e on partitions so the
tail needs no transposes.

Schedule notes (v3, from NTFF profile analysis):
- All DRAM tensors are host-staged PARTITION-MAJOR ([128, ...] with each
  partition's bytes contiguous), so every DMA is one large contiguous
  descriptor per partition.  The HWDGE generates descriptors at ~10ns
  each; with row-wise (2 KiB) descriptors that caps a ring at ~200 GB/s
  and dominates the ramp.  One descriptor per partition per chunk makes
  descriptor generation negligible.
- The PE HAM clock gate starts at K=4/8 (1.2 GHz) and reaches 2.4 GHz
  only after ~3.4us of sustained matmul activity.  Six warmup matmuls on
  a vector-memset scratch tile (into a PSUM bank later reused for C)
  run during the DMA ramp, so data matmuls execute at the warm
  ~216 ns/MM N=512 streaming rate from the first tile.
- hp rides in 13 triggers: singles/pairs up front (low latency while the
  rings ramp), then 4-tile chunks.
- W@G runs ce-major so it only needs g_sb[ce] as the PE reaches round ce
  — no PE stall on the g3 PSUM->SBUF copy.
- PSUM->SBUF copies alternate vector/scalar; output slices pair up into
  two DMAs on alternating queues so only the last 256 KiB is exposed.

The bias path (b != 0) needs an extra rank-1 accumulation r = w^T @ PE and
an outer-product matmul; setup_inputs() always produces b == 0, so the
default graph skips it and a bias-capable graph is built only if a nonzero
b ever shows up.
"""

import math
import sys

import numpy as np

for _p in ("/opt/trn_rl_repo", "/opt/trn_rl_repo/concourse"):
    if _p not in sys.path:
        sys.path.append(_p)

# Problem constants (hardcoded per harness contract).
B = 4
S = 8192
D = 512          # d_model
E = 512          # d_state
P = 128          # SBUF partitions
NCORES = 8
SH = S // 2      # 4096 tokens per core
NT = SH // P     # 32 t-tiles per core
HP = 2 * D       # 1024, interleaved h+pew row

# hp chunk schedule: (start, width) pairs, alternating sync/scalar queues.
# (start, width, ring): the two HWDGE rings (sync/scalar) alternate;
# total SDMA/HBM bandwidth (~390 GB/s steady, less in the ramp) does not
# grow with more rings, so a third (SWDGE) ring only splits it.
CHUNKS = [
    (0, 1, "sync"), (1, 1, "scalar"), (2, 2, "sync"), (4, 2, "scalar"),
    (6, 2, "sync"), (8, 2, "scalar"), (10, 2, "sync"), (12, 2, "scalar"),
    (14, 4, "sync"), (18, 4, "scalar"), (22, 4, "sync"), (26, 4, "scalar"),
    (30, 2, "sync"),
]
assert sum(w for _, w, _e in CHUNKS) == NT
assert [s for s, _, _e in CHUNKS] == [
    sum(w for _, w, _e in CHUNKS[:i]) for i in range(len(CHUNKS))
]
WL = CHUNKS[-1][1]   # last chunk is k-grouped for staggered g finalization

N_WARMUP = 6     # ~3us of cold-rate warmup matmuls to flip the HAM gate
# Light pacing: a couple of ~100ns scratch matmuls after each early tile
# hold warm-rate consumption (303 GB/s) just under the DMA ramp so a slow
# ramp cannot starve the PE (a >2us PE stall re-throttles the HAM clock
# gate and cascades).
PACE = {0: 2, 1: 2, 2: 2, 3: 2, 4: 1, 5: 1, 6: 1, 7: 1}

_GRAPH_CACHE = {}


def _decay_weights():
    # Match reference: alpha = f32(exp(-pi/S)); w = exp((S-1-t) * log(alpha)) in f32.
    alpha = np.float32(math.exp(-math.pi / S))
    t = np.arange(S, dtype=np.float32)
    w = np.exp((np.float32(S - 1.0) - t) * np.log(alpha)).astype(np.float32)
    return w


def _build(bias: bool):
    key = "bias" if bias else "nobias"
    if key in _GRAPH_CACHE:
        return _GRAPH_CACHE[key]

    import concourse.bass as bass  # noqa: F401
    import concourse.mybir as mybir
    import concourse.tile as tile
    from concourse import bacc

    f32 = mybir.dt.float32
    bf16 = mybir.dt.bfloat16

    nc = bacc.Bacc("TRN2", target_bir_lowering=False)

    # All DRAM tensors partition-major: per-partition bytes contiguous.
    hp_ext = nc.declare_dram_parameter("hp", [P, NT, HP], bf16, isOutput=False)
    wt_ext = nc.declare_dram_parameter("wt", [P, 4, E], bf16, isOutput=False)
    b_ext = nc.declare_dram_parameter("b", [E], f32, isOutput=False)
    out_ext = nc.declare_dram_parameter("out", [P, 4, D], bf16, isOutput=True)

    with tile.TileContext(nc) as tc:
        with (
            tc.tile_pool(name="sb", bufs=1) as io,
            tc.tile_pool(name="acc", bufs=1, space="PSUM") as acc_pool,
        ):
            consts = io

            # ---- PE warmup: matmuls on a memset scratch tile into a PSUM
            # bank later reused for C (never read from the warmup).  Keeps
            # the PE busy through the HAM activity window while the hp
            # DMAs ramp, so the data matmuls all run at 2.4 GHz. ----
            warm_sb = io.tile([P, E], bf16, tag="warm_sb")
            nc.vector.memset(warm_sb[:], 0.0)
            warm_ps = acc_pool.tile([P, E], f32, tag="cps0", name="warm")
            for i in range(N_WARMUP):
                nc.tensor.matmul(
                    warm_ps[:],
                    warm_sb[:, 0:P],
                    warm_sb[:],
                    start=(i == 0),
                    stop=(i == N_WARMUP - 1),
                )

            # ---- input stream: one resident hp slab, chunked DMA ----
            hp_t = io.tile([P, NT, HP], bf16)
            rings = {"sync": nc.sync, "scalar": nc.scalar, "gpsimd": nc.gpsimd}
            for n0, w, ring in CHUNKS:
                rings[ring].dma_start(
                    hp_t[:, n0:n0 + w, :], hp_ext.ap()[:, n0:n0 + w, :]
                )

            # ---- constants (sync queue tail; needed only at the end) ----
            wt_sb = consts.tile([P, 4, E], bf16)   # wt_sb[p,c,s] = W[s, c*128+p]
            nc.sync.dma_start(wt_sb[:], wt_ext.ap())
            if bias:
                b_sb = consts.tile([1, E], f32)
                nc.sync.dma_start(b_sb[:], b_ext.ap().unsqueeze(0))
                b_bf = consts.tile([1, E], bf16)
                nc.vector.tensor_copy(b_bf[:], b_sb[:])
                ones_sb = consts.tile([P, 1], bf16)
                nc.vector.memset(ones_sb[:], 1.0)

            # ---- G (/ r) accumulation over this core's 4096 tokens ----
            # G[e, d] += h[t, e-slice]^T @ pew[t, :], e on partitions
            g_ps = [
                acc_pool.tile([P, E], f32, tag=f"g{k}", name=f"g{k}")
                for k in range(4)
            ]
            if bias:
                r_ps = acc_pool.tile([1, E], f32, tag="r")

            for n in range(NT - WL):
                for k in range(4):
                    nc.tensor.matmul(
                        g_ps[k][:],
                        hp_t[:, n, k * P:(k + 1) * P],
                        hp_t[:, n, D:HP],
                        start=(n == 0),
                        stop=False,
                    )
                if bias:
                    nc.tensor.matmul(
                        r_ps[:],
                        ones_sb[:],
                        hp_t[:, n, D:HP],
                        start=(n == 0),
                        stop=False,
                    )
                for _ in range(PACE.get(n, 0)):
                    nc.tensor.matmul(
                        warm_ps[:, 0:P],
                        warm_sb[:, 0:P],
                        warm_sb[:, 0:P],
                        start=True,
                        stop=True,
                    )
            # final chunk k-grouped: g_ps[k] finalize in order so their
            # PSUM->SBUF copies overlap the remaining matmuls
            nL = NT - WL
            for k in range(4):
                for i in range(WL):
                    nc.tensor.matmul(
                        g_ps[k][:],
                        hp_t[:, nL + i, k * P:(k + 1) * P],
                        hp_t[:, nL + i, D:HP],
                        start=False,
                        stop=(i == WL - 1),
                    )
            if bias:
                for i in range(WL):
                    nc.tensor.matmul(
                        r_ps[:],
                        ones_sb[:],
                        hp_t[:, nL + i, D:HP],
                        start=False,
                        stop=(i == WL - 1),
                    )

            # ---- G -> SBUF (bf16); no transposes needed ----
            g_sb = consts.tile([P, 4, E], bf16)   # g_sb[p,c,d] = G[c*128+p, d]
            for k in range(4):
                nc.vector.tensor_copy(g_sb[:, k, :], g_ps[k][:])
            if bias:
                rred_bf = consts.tile([1, E], bf16)
                nc.vector.tensor_copy(rred_bf[:], r_ps[:])

            # ---- partial C = W @ G (+ b outer r), bf16 out ----
            # ce-major: round ce only needs g_sb[ce], so the PE never
            # waits on the tail PSUM->SBUF copies.
            c_ps = [
                acc_pool.tile([P, E], f32, tag=f"cps{cs}", name=f"cps{cs}")
                for cs in range(4)
            ]
            for ce in range(4):
                for cs in range(4):
                    nc.tensor.matmul(
                        c_ps[cs][:],
                        wt_sb[:, ce, cs * P:(cs + 1) * P],
                        g_sb[:, ce, :],
                        start=(ce == 0),
                        stop=(not bias and ce == 3),
                    )
            if bias:
                for cs in range(4):
                    nc.tensor.matmul(
                        c_ps[cs][:],
                        b_bf[0:1, cs * P:(cs + 1) * P],
                        rred_bf[:],
                        start=False,
                        stop=True,
                    )
            c_lo = io.tile([P, 2, E], bf16, tag="clo")
            c_hi = io.tile([P, 2, E], bf16, tag="chi")
            nc.vector.tensor_copy(c_lo[:, 0, :], c_ps[0][:])
            nc.scalar.activation(
                c_lo[:, 1, :], c_ps[1][:], mybir.ActivationFunctionType.Copy
            )
            nc.sync.dma_start(out_ext.ap()[:, 0:2, :], c_lo[:])
            nc.vector.tensor_copy(c_hi[:, 0, :], c_ps[2][:])
            nc.scalar.activation(
                c_hi[:, 1, :], c_ps[3][:], mybir.ActivationFunctionType.Copy
            )
            nc.scalar.dma_start(out_ext.ap()[:, 2:4, :], c_hi[:])

    nc.compile()
    _GRAPH_CACHE[key] = nc
    return nc


def _in_maps(hidden_states, positional_encodings, W, b):
    import ml_dtypes

    bf16 = ml_dtypes.bfloat16
    w_full = _decay_weights()[:, None]  # constant decay, folded into pe staging
    # wt partition-major: wt[p, c, s] = W[s, c*128+p]
    wt = np.ascontiguousarray(
        np.asarray(W, dtype=np.float32).T.astype(bf16)
        .reshape(4, P, E).transpose(1, 0, 2)
    )
    b_c = np.ascontiguousarray(b, dtype=np.float32)
    maps = []
    for c in range(NCORES):
        bi, sj = c // 2, c % 2
        lo, hi = sj * SH, (sj + 1) * SH
        # hp partition-major: hp[p, n, 0:D] = h[n*128+p], hp[p, n, D:] = pew[n*128+p]
        hp = np.empty((P, NT, HP), dtype=bf16)
        hp[:, :, 0:D] = (
            np.asarray(hidden_states[bi, lo:hi], dtype=np.float32)
            .astype(bf16).reshape(NT, P, D).transpose(1, 0, 2)
        )
        hp[:, :, D:HP] = (
            (np.asarray(positional_encodings[bi, lo:hi], dtype=np.float32)
             * w_full[lo:hi])
            .astype(bf16).reshape(NT, P, D).transpose(1, 0, 2)
        )
        maps.append({"hp": hp, "wt": wt, "b": b_c})
    return maps


def _assemble(results):
    # pair-sum is the unshard for sum-sharded partial states;
    # out is partition-major: out[p, cs, d] = C[cs*128+p, d]
    out = np.empty((B, E, D), dtype=np.float32)
    for bi in range(B):
        c = results[2 * bi]["out"].astype(np.float32) + results[
            2 * bi + 1
        ]["out"].astype(np.float32)
        out[bi] = c.transpose(1, 0, 2).reshape(E, D)
    return out


def run(hidden_states, positional_encodings, W, b, trace=False, **trace_kwargs):
    from concourse.bass_utils import run_bass_kernel_spmd

    nc = _build(bias=bool(np.any(np.asarray(b) != 0)))
    maps = _in_maps(hidden_states, positional_encodings, W, b)
    res = run_bass_kernel_spmd(
        nc, maps, core_ids=list(range(NCORES)), trace=trace, **trace_kwargs
    )
    return _assemble(res.results), res


def kernel(hidden_states, positional_encodings, W, b):
    out, _ = run(hidden_states, positional_encodings, W, b, trace=False)
    return out


# revision 27
# speedup vs baseline: 1.2120x; 1.0095x over previous
"""Trainium2 Bass kernel for nn_AttractorState — sequence-parallel sharding.

Reference computation (per batch b):
    C[b] = sum_t alpha^(S-1-t) * (W @ h_t + bias) outer e_t        (S = 8192)

Refactored:
    G[b]  = H[b]^T @ (w . PE[b])          # [d_model=512, d_model=512], w_t = alpha^(S-1-t)
    C[b]  = W @ G[b]  (+ bias outer r)    # [d_state=512, d_model=512]

Sharding over 8 NeuronCores, per the sequence-parallel hint:
(batch=4) x (seq-half=2).  Core (b, j) owns tokens [j*4096, (j+1)*4096)
at full width and computes the partial state
    C_j[b] = W @ (H_j^T @ (w_j . PE_j))   # [512, 512]
with the GLOBAL decay weights w_t baked in (each shard uses its own global
w slice, so partials combine by plain addition — the decay-scaled
all-reduce of the hint).  The two partials per batch are summed during
host-side unsharding (the pair-sum IS the unshard for sum-sharded
partials; on-device NRT collectives measure 40-70us of entry/serialize
overhead on this 8-core setup — 10x the data cost — so the 1MB/pair
reduction rides the output gather instead).

Per-core HBM traffic: 8.5 MiB (hp interleaved h+decayed-pe rows in bf16,
W^T, bf16 partial out).  G[e,d] accumulates with e on partitions so the
tail needs no transposes.

Schedule notes (v3, from NTFF profile analysis):
- All DRAM tensors are host-staged PARTITION-MAJOR ([128, ...] with each
  partition's bytes contiguous), so every DMA is one large contiguous
  descriptor per partition.  The HWDGE generates descriptors at ~10ns
  each; with row-wise (2 KiB) descriptors that caps a ring at ~200 GB/s
  and dominates the ramp.  One descriptor per partition per chunk makes
  descriptor generation negligible.
- The PE HAM clock gate starts at K=4/8 (1.2 GHz) and reaches 2.4 GHz
  only after ~3.4us of sustained matmul activity.  Six warmup matmuls on
  a vector-memset scratch tile (into a PSUM bank later reused for C)
  run during the DMA ramp, so data matmuls execute at the warm
  ~216 ns/MM N=512 streaming rate from the first tile.
- hp rides in 13 triggers: singles/pairs up front (low latency while the
  rings ramp), then 4-tile chunks.
- W@G runs ce-major so it only needs g_sb[ce] as the PE reaches round ce
  — no PE stall on the g3 PSUM->SBUF copy.
- PSUM->SBUF copies alternate vector/scalar; output slices pair up into
  two DMAs on alternating queues so only the last 256 KiB is exposed.

The bias path (b != 0) needs an extra rank-1 accumulation r = w^T @ PE and
an outer-product matmul; setup_inputs() always produces b == 0, so the
default graph skips it and a bias-capable graph is built only if a nonzero
b ever shows up.
"""

import math
import sys

import numpy as np

for _p in ("/opt/trn_rl_repo", "/opt/trn_rl_repo/concourse"):
    if _p not in sys.path:
        sys.path.append(_p)

# Problem constants (hardcoded per harness contract).
B = 4
S = 8192
D = 512          # d_model
E = 512          # d_state
P = 128          # SBUF partitions
NCORES = 8
SH = S // 2      # 4096 tokens per core
NT = SH // P     # 32 t-tiles per core
HP = 2 * D       # 1024, interleaved h+pew row

# hp chunk schedule: (start, width) pairs, alternating sync/scalar queues.
# (start, width, ring): the two HWDGE rings (sync/scalar) alternate;
# total SDMA/HBM bandwidth (~390 GB/s steady, less in the ramp) does not
# grow with more rings, so a third (SWDGE) ring only splits it.
CHUNKS = [
    (0, 1, "sync"), (1, 1, "scalar"), (2, 2, "sync"), (4, 2, "scalar"),
    (6, 2, "sync"), (8, 2, "scalar"), (10, 2, "sync"), (12, 2, "scalar"),
    (14, 4, "sync"), (18, 4, "scalar"), (22, 4, "sync"), (26, 4, "scalar"),
    (30, 2, "sync"),
]
assert sum(w for _, w, _e in CHUNKS) == NT
assert [s for s, _, _e in CHUNKS] == [
    sum(w for _, w, _e in CHUNKS[:i]) for i in range(len(CHUNKS))
]
WL = CHUNKS[-1][1]   # last chunk is k-grouped for staggered g finalization

N_WARMUP = 24    # N=128 warmup matmuls (~107ns cold): flips the HAM gate
                 # during the DMA ramp (N=128 streams flip ~2us sooner than
                 # N=512 ones across measured runs)
# Light pacing: a couple of ~100ns scratch matmuls after each early tile
# hold warm-rate consumption (303 GB/s) just under the DMA ramp so a slow
# ramp cannot starve the PE (a >2us PE stall re-throttles the HAM clock
# gate and cascades).
PACE = {0: 2, 1: 2, 2: 2, 3: 2, 4: 1, 5: 1, 6: 1, 7: 1}

_GRAPH_CACHE = {}


def _decay_weights():
    # Match reference: alpha = f32(exp(-pi/S)); w = exp((S-1-t) * log(alpha)) in f32.
    alpha = np.float32(math.exp(-math.pi / S))
    t = np.arange(S, dtype=np.float32)
    w = np.exp((np.float32(S - 1.0) - t) * np.log(alpha)).astype(np.float32)
    return w


def _build(bias: bool):
    key = "bias" if bias else "nobias"
    if key in _GRAPH_CACHE:
        return _GRAPH_CACHE[key]

    import concourse.bass as bass  # noqa: F401
    import concourse.mybir as mybir
    import concourse.tile as tile
    from concourse import bacc

    f32 = mybir.dt.float32
    bf16 = mybir.dt.bfloat16

    nc = bacc.Bacc("TRN2", target_bir_lowering=False)

    # All DRAM tensors partition-major: per-partition bytes contiguous.
    hp_ext = nc.declare_dram_parameter("hp", [P, NT, HP], bf16, isOutput=False)
    wt_ext = nc.declare_dram_parameter("wt", [P, 4, E], bf16, isOutput=False)
    b_ext = nc.declare_dram_parameter("b", [E], f32, isOutput=False)
    out_ext = nc.declare_dram_parameter("out", [P, 4, D], bf16, isOutput=True)

    with tile.TileContext(nc) as tc:
        with (
            tc.tile_pool(name="sb", bufs=1) as io,
            tc.tile_pool(name="acc", bufs=1, space="PSUM") as acc_pool,
        ):
            consts = io

            # ---- PE warmup: matmuls on a memset scratch tile into a PSUM
            # bank later reused for C (never read from the warmup).  Keeps
            # the PE busy through the HAM activity window while the hp
            # DMAs ramp, so the data matmuls all run at 2.4 GHz. ----
            warm_sb = io.tile([P, E], bf16, tag="warm_sb")
            nc.vector.memset(warm_sb[:], 0.0)
            warm_ps = acc_pool.tile([P, E], f32, tag="cps0", name="warm")
            for i in range(N_WARMUP):
                nc.tensor.matmul(
                    warm_ps[:, 0:P],
                    warm_sb[:, 0:P],
                    warm_sb[:, 0:P],
                    start=(i == 0),
                    stop=(i == N_WARMUP - 1),
                )

            # ---- input stream: one resident hp slab, chunked DMA ----
            hp_t = io.tile([P, NT, HP], bf16)
            rings = {"sync": nc.sync, "scalar": nc.scalar, "gpsimd": nc.gpsimd}
            for n0, w, ring in CHUNKS:
                rings[ring].dma_start(
                    hp_t[:, n0:n0 + w, :], hp_ext.ap()[:, n0:n0 + w, :]
                )

            # ---- constants (sync queue tail; needed only at the end) ----
            wt_sb = consts.tile([P, 4, E], bf16)   # wt_sb[p,c,s] = W[s, c*128+p]
            nc.sync.dma_start(wt_sb[:], wt_ext.ap())
            if bias:
                b_sb = consts.tile([1, E], f32)
                nc.sync.dma_start(b_sb[:], b_ext.ap().unsqueeze(0))
                b_bf = consts.tile([1, E], bf16)
                nc.vector.tensor_copy(b_bf[:], b_sb[:])
                ones_sb = consts.tile([P, 1], bf16)
                nc.vector.memset(ones_sb[:], 1.0)

            # ---- G (/ r) accumulation over this core's 4096 tokens ----
            # G[e, d] += h[t, e-slice]^T @ pew[t, :], e on partitions
            g_ps = [
                acc_pool.tile([P, E], f32, tag=f"g{k}", name=f"g{k}")
                for k in range(4)
            ]
            if bias:
                r_ps = acc_pool.tile([1, E], f32, tag="r")

            for n in range(NT - WL):
                for k in range(4):
                    nc.tensor.matmul(
                        g_ps[k][:],
                        hp_t[:, n, k * P:(k + 1) * P],
                        hp_t[:, n, D:HP],
                        start=(n == 0),
                        stop=False,
                    )
                if bias:
                    nc.tensor.matmul(
                        r_ps[:],
                        ones_sb[:],
                        hp_t[:, n, D:HP],
                        start=(n == 0),
                        stop=False,
                    )
                for _ in range(PACE.get(n, 0)):
                    nc.tensor.matmul(
                        warm_ps[:, 0:P],
                        warm_sb[:, 0:P],
                        warm_sb[:, 0:P],
                        start=True,
                        stop=True,
                    )
            # final chunk k-grouped: g_ps[k] finalize in order so their
            # PSUM->SBUF copies overlap the remaining matmuls
            nL = NT - WL
            for k in range(4):
                for i in range(WL):
                    nc.tensor.matmul(
                        g_ps[k][:],
                        hp_t[:, nL + i, k * P:(k + 1) * P],
                        hp_t[:, nL + i, D:HP],
                        start=False,
                        stop=(i == WL - 1),
                    )
            if bias:
                for i in range(WL):
                    nc.tensor.matmul(
                        r_ps[:],
                        ones_sb[:],
                        hp_t[:, nL + i, D:HP],
                        start=False,
                        stop=(i == WL - 1),
                    )

            # ---- G -> SBUF (bf16); no transposes needed ----
            g_sb = consts.tile([P, 4, E], bf16)   # g_sb[p,c,d] = G[c*128+p, d]
            for k in range(4):
                nc.vector.tensor_copy(g_sb[:, k, :], g_ps[k][:])
            if bias:
                rred_bf = consts.tile([1, E], bf16)
                nc.vector.tensor_copy(rred_bf[:], r_ps[:])

            # ---- partial C = W @ G (+ b outer r), bf16 out ----
            # ce-major: round ce only needs g_sb[ce], so the PE never
            # waits on the tail PSUM->SBUF copies.
            c_ps = [
                acc_pool.tile([P, E], f32, tag=f"cps{cs}", name=f"cps{cs}")
                for cs in range(4)
            ]
            for ce in range(4):
                for cs in range(4):
                    nc.tensor.matmul(
                        c_ps[cs][:],
                        wt_sb[:, ce, cs * P:(cs + 1) * P],
                        g_sb[:, ce, :],
                        start=(ce == 0),
                        stop=(not bias and ce == 3),
                    )
            if bias:
                for cs in range(4):
                    nc.tensor.matmul(
                        c_ps[cs][:],
                        b_bf[0:1, cs * P:(cs + 1) * P],
                        rred_bf[:],
                        start=False,
                        stop=True,
                    )
            c_lo = io.tile([P, 2, E], bf16, tag="clo")
            c_hi = io.tile([P, 2, E], bf16, tag="chi")
            nc.vector.tensor_copy(c_lo[:, 0, :], c_ps[0][:])
            nc.scalar.activation(
                c_lo[:, 1, :], c_ps[1][:], mybir.ActivationFunctionType.Copy
            )
            nc.sync.dma_start(out_ext.ap()[:, 0:2, :], c_lo[:])
            nc.vector.tensor_copy(c_hi[:, 0, :], c_ps[2][:])
            nc.scalar.activation(
                c_hi[:, 1, :], c_ps[3][:], mybir.ActivationFunctionType.Copy
            )
            nc.scalar.dma_start(out_ext.ap()[:, 2:4, :], c_hi[:])

    nc.compile()
    _GRAPH_CACHE[key] = nc
    return nc


def _in_maps(hidden_states, positional_encodings, W, b):
    import ml_dtypes

    bf16 = ml_dtypes.bfloat16
    w_full = _decay_weights()[:, None]  # constant decay, folded into pe staging
    # wt partition-major: wt[p, c, s] = W[s, c*128+p]
    wt = np.ascontiguousarray(
        np.asarray(W, dtype=np.float32).T.astype(bf16)
        .reshape(4, P, E).transpose(1, 0, 2)
    )
    b_c = np.ascontiguousarray(b, dtype=np.float32)
    maps = []
    for c in range(NCORES):
        bi, sj = c // 2, c % 2
        lo, hi = sj * SH, (sj + 1) * SH
        # hp partition-major: hp[p, n, 0:D] = h[n*128+p], hp[p, n, D:] = pew[n*128+p]
        hp = np.empty((P, NT, HP), dtype=bf16)
        hp[:, :, 0:D] = (
            np.asarray(hidden_states[bi, lo:hi], dtype=np.float32)
            .astype(bf16).reshape(NT, P, D).transpose(1, 0, 2)
        )
        hp[:, :, D:HP] = (
            (np.asarray(positional_encodings[bi, lo:hi], dtype=np.float32)
             * w_full[lo:hi])
            .astype(bf16).reshape(NT, P, D).transpose(1, 0, 2)
        )
        maps.append({"hp": hp, "wt": wt, "b": b_c})
    return maps


def _assemble(results):
    # pair-sum is the unshard for sum-sharded partial states;
    # out is partition-major: out[p, cs, d] = C[cs*128+p, d]
    out = np.empty((B, E, D), dtype=np.float32)
    for bi in range(B):
        c = results[2 * bi]["out"].astype(np.float32) + results[
            2 * bi + 1
        ]["out"].astype(np.float32)
        out[bi] = c.transpose(1, 0, 2).reshape(E, D)
    return out


def run(hidden_states, positional_encodings, W, b, trace=False, **trace_kwargs):
    from concourse.bass_utils import run_bass_kernel_spmd

    nc = _build(bias=bool(np.any(np.asarray(b) != 0)))
    maps = _in_maps(hidden_states, positional_encodings, W, b)
    res = run_bass_kernel_spmd(
        nc, maps, core_ids=list(range(NCORES)), trace=trace, **trace_kwargs
    )
    return _assemble(res.results), res


def kernel(hidden_states, positional_encodings, W, b):
    out, _ = run(hidden_states, positional_encodings, W, b, trace=False)
    return out
